# revision 1
# baseline (speedup 1.0000x reference)
"""Trainium2 Bass kernel for the BlockDiagonalACDC layer (parity-split L1).

out = riffle(idct2(gconv(dct2(gconv(x, A)), D))) + bias, all linear along
the feature dim (4096). DCT-II parity symmetry halves both dense passes:
  fwd:  u± = z1[:2048] ± reverse(z1[2048:]); z2_even = u+ @ E1,
        z2_odd = u- @ O1  (E1/O1 = parity column slices of Ct[:2048]).
        The reversal is free: groups >= 16 of gconv(A) run with a
        free-dim-reversed lhsT so their PSUM comes out partition-reversed.
  gconv(D): conjugated into parity-block space (block-diag quadrant lhsT
        tiles built on device from D).
  inv:  s = z3_even @ GmE, t = z3_odd @ GmO with columns pre-permuted so
        riffle+reversal land as contiguous output runs; out = s±t, bias
        injected as half-sum/diff rows via K=1 matmuls into PSUM.

Sharding: pure data parallel, 2048 batch rows per core on 8 cores.
"""

import numpy as np
import ml_dtypes

import concourse.bacc as bacc
import concourse.mybir as mybir
from concourse.tile import TileContext
from concourse.bass_utils import run_bass_kernel_spmd
from concourse.masks import make_identity

N_BATCH, D_FEAT, GROUPS = 16384, 4096, 32
N_CORES = 8
N_SHARD = N_BATCH // N_CORES      # 2048 rows per core
CHUNK = 512                       # batch rows per pipeline chunk
N_CHUNKS = N_SHARD // CHUNK       # 4
FTILES = D_FEAT // 128            # 32
HT = FTILES // 2                  # 16
QW = 512                          # inverse strip width
NQ = 2048 // QW                   # 4 strips

_BF16 = mybir.dt.bfloat16
_F32 = mybir.dt.float32

# output runs per inverse strip c (args q in [QW*c, QW*(c+1))):
#   s+t -> ascending from PLUS_START[c]; s-t -> reversed into MINUS_LO[c]..
PLUS_START = [QW * c if c < NQ // 2 else 1024 + QW * c for c in range(NQ)]
MINUS_LO = [(4096 - QW) - QW * c if c < NQ // 2 else (3072 - QW) - QW * c
            for c in range(NQ)]


def _host_constants():
    N = D_FEAT
    H = N // 2
    j = np.arange(N, dtype=np.float64)
    k = np.arange(N, dtype=np.float64)[:, None]
    ang = np.pi * k * (2.0 * j[None, :] + 1.0) / (2.0 * N)
    C = 2.0 * np.cos(ang)
    Ct = np.ascontiguousarray(C.T)                    # [j, k]
    w = np.ones(N); w[0] = 0.5
    Gm = (1.0 / N) * w[:, None] * np.cos(ang)         # [k, j]

    E1 = Ct[:H, 0::2]      # [2048, 2048]
    O1 = Ct[:H, 1::2]

    def tile_fwd(M):       # [tau, p, fc, m]
        return np.ascontiguousarray(
            M.reshape(HT, 128, HT, 128).transpose(2, 1, 0, 3))
    fwd_host = np.stack([tile_fwd(E1), tile_fwd(O1)]).astype(ml_dtypes.bfloat16)

    cols = np.concatenate([np.arange(0, H, 2), np.arange(1, H, 2)])
    GmE = Gm[0::2][:, :H][:, cols]    # [2048 m, 2048 q]
    GmO = Gm[1::2][:, :H][:, cols]

    def tile_inv(M):       # [qc, p, kc, q]
        return np.ascontiguousarray(
            M.reshape(HT, 128, NQ, QW).transpose(2, 1, 0, 3))
    inv_host = np.stack([tile_inv(GmE), tile_inv(GmO)]).astype(ml_dtypes.bfloat16)

    out_plus = np.where(cols % 2 == 0, cols // 2, 2048 + (cols - 1) // 2)
    jm = 4095 - cols
    out_minus = np.where(jm % 2 == 0, jm // 2, 2048 + (jm - 1) // 2)
    return fwd_host, inv_host, out_plus.astype(np.int64), out_minus.astype(np.int64)


def _build_program(reps=1):
    nc = bacc.Bacc()
    # xs pre-transposed on host to [128, 16, 4096] so row-tile loads batch
    xs = nc.dram_tensor("xs", (128, N_SHARD // 128, D_FEAT), _F32,
                        kind="ExternalInput")
    # A/D pre-transposed to [128, g, 128] so the full weight loads in one DMA
    Aw = nc.dram_tensor("Aw", (128, GROUPS, 128), _F32, kind="ExternalInput")
    Dw = nc.dram_tensor("Dw", (128, GROUPS, 128), _F32, kind="ExternalInput")
    bias_s = nc.dram_tensor("bias_s", (1, 2048), _F32, kind="ExternalInput")
    bias_t = nc.dram_tensor("bias_t", (1, 2048), _F32, kind="ExternalInput")
    fwdw = nc.dram_tensor("fwdw", (2, HT, 128, HT, 128), _BF16, kind="ExternalInput")
    invw = nc.dram_tensor("invw", (2, NQ, 128, HT, QW), _BF16, kind="ExternalInput")
    out = nc.dram_tensor("out", (N_SHARD, D_FEAT), _F32, kind="ExternalOutput")

    with TileContext(nc) as tc:
        with (
            tc.tile_pool(name="const", bufs=1) as constp,
            tc.tile_pool(name="stage", bufs=3) as stagep,
            tc.tile_pool(name="xbf", bufs=2) as xbfp,
            tc.tile_pool(name="fwp", bufs=3) as fwp,
            tc.tile_pool(name="ivp", bufs=4) as ivp,
            tc.tile_pool(name="ost", bufs=3) as ostp,
            tc.tile_pool(name="mm_ps", bufs=3, space="PSUM") as mmp,
            tc.tile_pool(name="tp_ps", bufs=2, space="PSUM") as tpp,
            tc.tile_pool(name="st_ps", bufs=3, space="PSUM") as stp,
        ):
            # weight loads first so PE's AT/LT transposes start ASAP
            awbf = stagep.tile([128, D_FEAT], _BF16, tag="stage")
            nc.gpsimd.dma_start(awbf[:], Aw[:])
            dwbf_early = stagep.tile([128, D_FEAT], _BF16, tag="stage")
            nc.gpsimd.dma_start(dwbf_early[:], Dw[:])
            ident = constp.tile([128, 128], _BF16, tag="ident")
            make_identity(nc, ident[:])
            ones1 = constp.tile([1, 128], _BF16, tag="ones1")
            nc.gpsimd.memset(ones1[:], 1.0)
            bs_bf = constp.tile([1, 2048], _BF16, tag="bs")
            bt_bf = constp.tile([1, 2048], _BF16, tag="bt")
            nc.gpsimd.dma_start(bs_bf[:], bias_s[:])
            nc.gpsimd.dma_start(bt_bf[:], bias_t[:])

            # ---- A weights: AT[g] = A[g].T
            AT = constp.tile([128, D_FEAT], _BF16, tag="AT")
            for g4 in range(GROUPS // 4):
                # setup transposes run through mmp (3 bufs, idle during
                # setup) so PE isn't throttled by tpp's 2-buf depth
                ps = mmp.tile([128, 512], _BF16, tag="mm")
                for gg in range(4):
                    g = g4 * 4 + gg
                    nc.tensor.transpose(
                        ps[:, gg * 128:(gg + 1) * 128],
                        awbf[:, g * 128:(g + 1) * 128], ident[:])
                for gg in range(4):
                    g = g4 * 4 + gg
                    sl = slice(g * 128, (g + 1) * 128)
                    psl = slice(gg * 128, (gg + 1) * 128)
                    ceng = (nc.vector.tensor_copy, nc.scalar.copy)[gg % 2]
                    if g < 16:
                        ceng(AT[:, sl], ps[:, psl])
                    else:
                        # store columns reversed: gconvA for groups >= 16
                        # then emits partition-reversed (butterfly-ready) tiles
                        ceng(AT[:, sl],
                             ps[:, (gg + 1) * 128 - 1:gg * 128 - 1 if gg else None:-1])

            # ---- D weights: conjugated quadrant tiles LT[x][y].
            # Tiles + memsets here; the transpose/copy build is deferred into
            # chunk 0 (after gconvA) so its DVE/Act copy chains overlap the
            # first chunk's dense PE work instead of stalling PE upfront.
            dwbf = dwbf_early
            LT = [[constp.tile([128, HT * 128], _BF16, tag=f"LT{x}{y}",
                                name=f"LT{x}{y}")
                   for y in range(2)] for x in range(2)]
            for x in range(2):
                for y in range(2):
                    nc.gpsimd.memset(LT[x][y][:], 0.0)

            def build_lt():
                for tau in range(HT):
                    g1 = 2 * tau
                    for x in range(2):
                        ps = mmp.tile([128, 512], _BF16, tag="mm")
                        nc.tensor.transpose(
                            ps[:, 0:128],
                            dwbf[:, g1 * 128 + x:(g1 + 2) * 128:2],
                            ident[:])
                        for y in range(2):
                            ca = (nc.vector.tensor_copy, nc.scalar.copy)[y]
                            cb = (nc.scalar.copy, nc.vector.tensor_copy)[y]
                            ca(LT[x][y][0:64, tau * 128:tau * 128 + 64],
                               ps[0:64, y:128:2])
                            cb(LT[x][y][64:128, tau * 128 + 64:(tau + 1) * 128],
                               ps[64:128, y:128:2])

            build_lt()

            rep_ctx = tc.For_i(0, reps, 1) if reps > 1 else None
            if rep_ctx is not None:
                rep_ctx.__enter__()
            for ci in range(N_CHUNKS):
                r0 = ci * CHUNK
                # ---- transpose-in
                xT = stagep.tile([128, FTILES * CHUNK], _BF16, tag="stage")
                nt0 = r0 // 128
                for ntp in range(2):
                    xbfs = []
                    for nn in range(2):
                        xbf = xbfp.tile([128, D_FEAT], _BF16, tag="xbf")
                        nc.gpsimd.dma_start(
                            xbf[:], xs[:, nt0 + ntp * 2 + nn, :])
                        xbfs.append(xbf)
                    for fc in range(FTILES):
                        ps = tpp.tile([128, 512], _BF16, tag="tp")
                        for nn in range(2):
                            nc.tensor.transpose(
                                ps[:, nn * 128:(nn + 1) * 128],
                                xbfs[nn][:, fc * 128:(fc + 1) * 128], ident[:])
                        eng = nc.vector if fc % 2 else nc.scalar
                        (eng.tensor_copy if eng is nc.vector else eng.copy)(
                            xT[:, fc * CHUNK + ntp * 256:
                               fc * CHUNK + ntp * 256 + 256],
                            ps[:, 0:256])
                # ---- gconvA; groups >= 16 with reversed lhsT -> stored
                # at slot 47-g as reversed tiles (butterfly-ready)
                z1 = stagep.tile([128, FTILES * CHUNK], _BF16, tag="stage")
                for g in range(GROUPS):
                    ps = mmp.tile([128, CHUNK], _F32, tag="mm")
                    nc.tensor.matmul(
                        ps[:], AT[:, g * 128:(g + 1) * 128],
                        xT[:, g * CHUNK:(g + 1) * CHUNK],
                        start=True, stop=True)
                    slot = g if g < 16 else 47 - g
                    if g % 2:
                        nc.scalar.copy(z1[:, slot * CHUNK:(slot + 1) * CHUNK], ps[:])
                    else:
                        nc.vector.tensor_copy(
                            z1[:, slot * CHUNK:(slot + 1) * CHUNK], ps[:])

                # ---- butterfly: uu = [up tiles 0..15 | um tiles 16..31]
                uu = stagep.tile([128, FTILES * CHUNK], _BF16, tag="stage")
                for t in range(HT):
                    lo = slice(t * CHUNK, (t + 1) * CHUNK)
                    hi = slice((16 + t) * CHUNK, (17 + t) * CHUNK)
                    nc.vector.tensor_add(uu[:, lo], z1[:, lo], z1[:, hi])
                    nc.vector.tensor_sub(uu[:, hi], z1[:, lo], z1[:, hi])

                # ---- fwd dense: z2 = [E-block | O-block]
                z2 = stagep.tile([128, FTILES * CHUNK], _BF16, tag="stage")
                for b in range(2):
                    for tau in range(HT):
                        fw = fwp.tile([128, HT, 128], _BF16, tag="fw")
                        nc.sync.dma_start(fw[:], fwdw[b, tau])
                        ps = mmp.tile([128, CHUNK], _F32, tag="mm")
                        for fc in range(HT):
                            nc.tensor.matmul(
                                ps[:], fw[:, fc, :],
                                uu[:, (16 * b + fc) * CHUNK:
                                   (16 * b + fc + 1) * CHUNK],
                                start=(fc == 0), stop=(fc == HT - 1))
                        slot = 16 * b + tau
                        if tau % 2:
                            nc.scalar.copy(
                                z2[:, slot * CHUNK:(slot + 1) * CHUNK], ps[:])
                        else:
                            nc.vector.tensor_copy(
                                z2[:, slot * CHUNK:(slot + 1) * CHUNK], ps[:])

                # ---- conjugated gconvD
                z3 = stagep.tile([128, FTILES * CHUNK], _BF16, tag="stage")
                for y in range(2):
                    for tau in range(HT):
                        ps = mmp.tile([128, CHUNK], _F32, tag="mm")
                        for x in range(2):
                            nc.tensor.matmul(
                                ps[:], LT[x][y][:, tau * 128:(tau + 1) * 128],
                                z2[:, (16 * x + tau) * CHUNK:
                                   (16 * x + tau + 1) * CHUNK],
                                start=(x == 0), stop=(x == 1))
                        slot = 16 * y + tau
                        if tau % 2:
                            nc.scalar.copy(
                                z3[:, slot * CHUNK:(slot + 1) * CHUNK], ps[:])
                        else:
                            nc.vector.tensor_copy(
                                z3[:, slot * CHUNK:(slot + 1) * CHUNK], ps[:])

                # ---- inverse strips + butterfly + bias + store
                for qc in range(NQ):
                    ivs = []
                    for b in range(2):
                        for h in range(2):
                            iv = ivp.tile([128, HT // 2, QW], _BF16, tag="iv")
                            nc.sync.dma_start(
                                iv[:], invw[b, qc, :, h * (HT // 2):(h + 1) * (HT // 2)])
                            ivs.append(iv)
                    for nt in range(CHUNK // 128):
                        pst = []
                        for b in range(2):
                            ps = stp.tile([128, QW], _F32, tag="st")
                            for kc in range(HT):
                                nc.tensor.matmul(
                                    ps[:],
                                    z3[:, (16 * b + kc) * CHUNK + nt * 128:
                                       (16 * b + kc) * CHUNK + (nt + 1) * 128],
                                    ivs[2 * b + kc // 8][:, kc % 8, :],
                                    start=(kc == 0), stop=False)
                            brow = bs_bf if b == 0 else bt_bf
                            nc.tensor.matmul(
                                ps[:], ones1[:],
                                brow[0:1, qc * QW:(qc + 1) * QW],
                                start=False, stop=True)
                            pst.append(ps)
                        tsb = ostp.tile([128, QW], _F32, tag="ost")
                        nc.scalar.copy(tsb[:], pst[1][:])
                        op = ostp.tile([128, QW], _F32, tag="ost")
                        om = ostp.tile([128, QW], _F32, tag="ost")
                        nc.vector.tensor_add(op[:], pst[0][:], tsb[:])
                        nc.vector.tensor_sub(om[:, ::-1], pst[0][:], tsb[:])
                        rows = slice(r0 + nt * 128, r0 + (nt + 1) * 128)
                        nc.sync.dma_start(
                            out[rows, PLUS_START[qc]:PLUS_START[qc] + QW], op[:])
                        nc.sync.dma_start(
                            out[rows, MINUS_LO[qc]:MINUS_LO[qc] + QW],
                            om[:])
            if rep_ctx is not None:
                rep_ctx.__exit__(None, None, None)
    nc.finalize()
    return nc


_CACHE = {}


def kernel(x, A, D, bias):
    if "nc" not in _CACHE:
        _CACHE["consts"] = _host_constants()
        _CACHE["nc"] = _build_program()
    nc = _CACHE["nc"]
    fwd_host, inv_host, out_plus, out_minus = _CACHE["consts"]

    bias_v = np.asarray(bias, dtype=np.float64).reshape(-1)
    bs = ((bias_v[out_plus] + bias_v[out_minus]) / 2).astype(np.float32)[None]
    bt = ((bias_v[out_plus] - bias_v[out_minus]) / 2).astype(np.float32)[None]

    x = np.ascontiguousarray(x, dtype=np.float32)
    At = np.ascontiguousarray(
        np.asarray(A, dtype=np.float32).transpose(1, 0, 2))
    Dt = np.ascontiguousarray(
        np.asarray(D, dtype=np.float32).transpose(1, 0, 2))
    in_maps = []
    for c in range(N_CORES):
        shard = x[c * N_SHARD:(c + 1) * N_SHARD]
        xs_t = np.ascontiguousarray(
            shard.reshape(N_SHARD // 128, 128, D_FEAT).transpose(1, 0, 2))
        in_maps.append({
            "xs": xs_t,
            "Aw": At, "Dw": Dt,
            "bias_s": bs, "bias_t": bt,
            "fwdw": fwd_host, "invw": inv_host,
        })
    res = run_bass_kernel_spmd(nc, in_maps, core_ids=list(range(N_CORES)))
    return np.concatenate([res.results[c]["out"] for c in range(N_CORES)], axis=0)



# revision 66
# speedup vs baseline: 1.0216x; 1.0216x over previous
"""Trainium2 Bass kernel for BlockDiagonalACDC — depth-2 DCT factorization.

out = riffle(idct2(gconv(dct2(gconv(x, A)), D))) + bias.

The 4096-point DCT-II is factored two levels deep:
  L1 fold:  u = f[:2048]+rev(f[2048:]), v = f[:2048]-rev(f[2048:])
  L2 fold:  ulo/uhi from u;  Givens rotations (p, qt) from v
  leaves:   four dense 1024x1024 matmuls (DCT2/DCT4/DCT2/DCT2)
  lift:     odd-k outputs are +-1 pairs of leaf2/leaf3 outputs, folded with
            gconv(D) and the idct input diag into one middle matrix Meff
            (applied as ~140 sparse 128x128 blocks).
  inverse:  transposed leaves (data-stationary), then transposed rotations /
            folds on [batch, j] tiles with broadcast coeff rows; riffle and
            the final reversal land as 8 contiguous 512-wide output runs.

z1 orientation trick: gconvA lhsT column-reversed for tiles 8-15/24-31 so all
fold/rotation pairings are partition-aligned elementwise DVE ops.

Sharding: pure data parallel, 2048 batch rows per core on 8 cores.
"""

import numpy as np
import ml_dtypes

import concourse.bacc as bacc
import concourse.mybir as mybir
from concourse.tile import TileContext
from concourse.bass_utils import run_bass_kernel_spmd
from concourse.masks import make_identity

N_BATCH, D_FEAT, GROUPS = 16384, 4096, 32
N_CORES = 8
N_SHARD = N_BATCH // N_CORES      # 2048 rows per core
CHUNK = 512
N_CHUNKS = N_SHARD // CHUNK       # 4
FTILES = D_FEAT // 128            # 32
N, H, Q = 4096, 2048, 1024

_BF16 = mybir.dt.bfloat16
_F32 = mybir.dt.float32
_MUL = mybir.AluOpType.mult
_ADD = mybir.AluOpType.add

# inverse output runs (512 wide): [strip][slot fa..fd] -> (start, reversed)
# strip 0 = even j' (asc), strip 1 = odd j' (colperm order)
RUN = [[(0, False), (3584, True), (2560, True), (1024, False)],
       [(2048, False), (1536, True), (512, True), (3072, False)]]
QW = 512
NSTRIP = 2


def _dct2m(n):
    k = np.arange(n)[:, None]; j = np.arange(n)[None, :]
    return np.cos(np.pi * k * (2 * j + 1) / (2 * n))


def _dct4m(n):
    m = np.arange(n)[:, None]; j = np.arange(n)[None, :]
    return np.cos(np.pi * (2 * m + 1) * (2 * j + 1) / (4 * n))


def _dst_sign():
    h = 8
    r = np.arange(h)[:, None]; j = np.arange(h)[None, :]
    d2 = np.cos(np.pi * r * (2 * j + 1) / (2 * h))
    s2 = np.sin(np.pi * (r + 1) * (2 * j + 1) / (2 * h))
    q = np.random.RandomState(0).randn(h)
    qt = q * np.where(np.arange(h) % 2 == 0, 1.0, -1.0)
    b = s2 @ q
    return 1.0 if np.allclose(b, (d2 @ qt)[::-1]) else -1.0


def _host_static():
    """A/D/bias-independent constants."""
    dst_sign = _dst_sign()
    # leaf3 col r holds B[r-1] = dst_sign*2*dct2(qt)[1024-r] (col 0: B[1023])
    # so lift pairs (leaf2 col r, leaf3 col r) are tile-aligned
    l3rows = np.r_[0, np.arange(1023, 0, -1)]
    leaf = [2.0 * _dct2m(Q), 2.0 * _dct4m(Q), 2.0 * _dct2m(Q),
            2.0 * _dct2m(Q)[l3rows]]

    alpha = (2 * np.arange(Q) + 1) * np.pi / (4 * H)
    rc, rs = np.cos(alpha), np.sin(alpha)
    sgn = np.where(np.arange(Q) % 2 == 0, 1.0, -1.0)
    qa, qb = -sgn * rs, sgn * rc

    # lift table: k -> up to 2 (leaf-flat index a, sign)
    A1 = np.zeros(N, np.int64); S1 = np.zeros(N)
    A2 = np.zeros(N, np.int64); S2 = np.zeros(N)
    for r in range(Q):
        A1[4 * r], S1[4 * r] = 0 * Q + r, 1.0
        A1[4 * r + 2], S1[4 * r + 2] = 1 * Q + r, 1.0
    A1[1], S1[1] = 2 * Q + 0, 1.0
    for r in range(1, Q):
        # leaf3 col r = B[r-1] (dst_sign folded into lift signs below)
        A1[4 * r - 1], S1[4 * r - 1] = 2 * Q + r, 1.0
        A2[4 * r - 1], S2[4 * r - 1] = 3 * Q + r, -dst_sign
        A1[4 * r + 1], S1[4 * r + 1] = 2 * Q + r, 1.0
        A2[4 * r + 1], S2[4 * r + 1] = 3 * Q + r, dst_sign
    A1[4 * Q - 1], S1[4 * Q - 1] = 3 * Q + 0, -dst_sign

    # fwd leaf weights: fwdw[L, m, i, tau, c] = leaf[L][128m+c, 128tau+i]
    fwdw = np.stack([
        M.T.reshape(8, 128, 8, 128).transpose(2, 1, 0, 3) for M in leaf
    ]).astype(ml_dtypes.bfloat16)

    # inverse: stage-col parity permutation, per strip
    colperm = np.concatenate([np.arange(0, Q, 2), np.arange(1, Q, 2)])
    # invw[strip, wave, i, b, kt, c] =
    #     (leaf[2*wave+b]/2)[128kt+i, colperm[512*strip+c]]
    pieces = [(M / 2.0)[:, colperm].reshape(8, 128, 2, 512) for M in leaf]
    invw = np.zeros((2, 2, 128, 2, 8, 512), np.float32)
    for L in range(4):
        # pieces[L][kt, i, s, c] -> invw[s, L//2, i, L%2, kt, c]
        invw[:, L // 2, :, L % 2, :, :] = pieces[L].transpose(2, 1, 0, 3)
    invw = invw.astype(ml_dtypes.bfloat16)

    # fwd rotation per-partition coeffs cfw[p, cf, t'] (cf: c,s,qa,qb)
    cfw = np.stack([rc, rs, qa, qb]).reshape(4, 8, 128).transpose(2, 0, 1)
    cfw = np.ascontiguousarray(cfw).astype(np.float32)

    # inverse rotation free-dim coeffs ciw[p, cf, strip, c] (broadcast rows)
    ci = np.stack([rc, rs, qa, qb])[:, colperm].reshape(4, 2, 512)
    ciw = np.broadcast_to(ci, (128, 4, 2, 512)).astype(ml_dtypes.bfloat16)
    ciw = np.ascontiguousarray(ciw)

    # Meff block sparsity pattern (structure only): big blocks are K=128
    # matmuls; sparse spill blocks decompose into <=32-row pieces (K=32).
    rng = np.random.RandomState(0)
    Mp = _build_meff(rng.randn(GROUPS, 128, 128), A1, S1, A2, S2)
    big = []     # (kt, mt)
    small = []   # (kt, mt, w): 32-row window w (w=3 runs as K=64 from base 64)
    for kt in range(32):
        for mt in range(32):
            blk = Mp[kt * 128:(kt + 1) * 128, mt * 128:(mt + 1) * 128]
            rows = np.where(np.abs(blk).max(axis=1) > 1e-12)[0]
            if len(rows) == 0:
                continue
            if len(rows) > 48:
                big.append((kt, mt))
                continue
            for w in sorted(set(int(r) // 32 for r in rows)):
                small.append((kt, mt, w))
    # pack small pieces into tile columns by window class; window 3 runs as
    # K=64 from base 64 so it must not share a column with a window-2 piece.
    cls = {w: [p for p in small if p[2] == w] for w in range(4)}
    ncol = max(len(cls[0]), len(cls[1]), len(cls[2]) + len(cls[3]), 1)
    packed = []  # (kt, mt, w, col)
    for w in (0, 1):
        for j, (kt, mt, _) in enumerate(cls[w]):
            packed.append((kt, mt, w, j))
    for j, (kt, mt, _) in enumerate(cls[2]):
        packed.append((kt, mt, 2, j))
    for j, (kt, mt, _) in enumerate(cls[3]):
        packed.append((kt, mt, 3, len(cls[2]) + j))

    return dict(leaf=leaf, fwdw=fwdw, invw=invw, cfw=cfw, ciw=ciw,
                colperm=colperm, lift=(A1, S1, A2, S2), big=big,
                packed=packed, ncol=ncol, rot=(rc, rs, qa, qb))


def _build_meff(D, A1, S1, A2, S2):
    w = np.ones(N); w[0] = 0.5
    wN = w / N
    M = np.zeros((N, N))
    for g in range(GROUPS):
        kk = np.arange(128) + 128 * g
        blk = D[g].T * wN[kk][None, :]        # [i, o]
        for (ai, si) in ((A1[kk], S1[kk]), (A2[kk], S2[kk])):
            for (ao, so) in ((A1[kk], S1[kk]), (A2[kk], S2[kk])):
                contrib = (si[:, None] * so[None, :]) * blk
                np.add.at(M, (ai[:, None], ao[None, :]), contrib)
    return M


def _build_program(reps=1):
    st = _prep()
    nbig = len(st["big"])
    nsmt = st["ncol"]
    nc = bacc.Bacc()
    xs = nc.dram_tensor("xs", (128, N_SHARD // 128, D_FEAT), _F32,
                        kind="ExternalInput")
    atv = nc.dram_tensor("atv", (128, FTILES, 128), _BF16, kind="ExternalInput")
    fwdw = nc.dram_tensor("fwdw", (4, 8, 128, 8, 128), _BF16,
                          kind="ExternalInput")
    meffw = nc.dram_tensor("meffw", (128, nbig, 128), _BF16,
                           kind="ExternalInput")
    smallw = nc.dram_tensor("smallw", (128, max(nsmt, 1), 128), _BF16,
                            kind="ExternalInput")
    invw = nc.dram_tensor("invw", (2, 2, 128, 2, 8, 512), _BF16,
                          kind="ExternalInput")
    cfw = nc.dram_tensor("cfw", (128, 4, 8), _F32, kind="ExternalInput")
    ciw = nc.dram_tensor("ciw", (128, 4, 2, 512), _BF16, kind="ExternalInput")
    bstw = nc.dram_tensor("bstw", (1, 4, 2, 512), _BF16, kind="ExternalInput")
    out = nc.dram_tensor("out", (N_SHARD, D_FEAT), _F32, kind="ExternalOutput")

    with TileContext(nc) as tc:
        with (
            tc.tile_pool(name="const", bufs=1) as constp,
            tc.tile_pool(name="stage", bufs=3) as stagep,
            tc.tile_pool(name="xbf", bufs=2) as xbfp,
            tc.tile_pool(name="fwp", bufs=2) as fwp,
            tc.tile_pool(name="mfp", bufs=2) as mfp,
            tc.tile_pool(name="ivp", bufs=2) as ivp,
            tc.tile_pool(name="tmp", bufs=5) as tmpp,
            tc.tile_pool(name="ost", bufs=5) as ostp,
            tc.tile_pool(name="mm_ps", bufs=2, space="PSUM") as mmp,
            tc.tile_pool(name="tp_ps", bufs=2, space="PSUM") as tpp,
            tc.tile_pool(name="st_ps", bufs=4, space="PSUM") as stp,
        ):
            ident = constp.tile([128, 128], _BF16, tag="ident")
            make_identity(nc, ident[:])
            ones1 = constp.tile([1, 128], _BF16, tag="ones1")
            nc.gpsimd.memset(ones1[:], 1.0)
            atv_t = constp.tile([128, FTILES, 128], _BF16, tag="atv")
            nc.scalar.dma_start(atv_t[:], atv[:])
            small_t = constp.tile([128, max(nsmt, 1), 128], _BF16, tag="smeff")
            nc.scalar.dma_start(small_t[:], smallw[:])
            cf_t = constp.tile([128, 4, 8], _F32, tag="cf")
            nc.scalar.dma_start(cf_t[:], cfw[:])
            ci_t = constp.tile([128, 4, 2, 512], _BF16, tag="ci")
            nc.scalar.dma_start(ci_t[:], ciw[:])
            bst_t = constp.tile([1, 4, 2, 512], _BF16, tag="bst")
            nc.scalar.dma_start(bst_t[:], bstw[:])

            # middle-matmul emission lists per output tile; big blocks are
            # stored in meffw sorted by mt (kernel() packs them identically)
            by_mt = {mt: [] for mt in range(32)}
            off = 0
            mt_off = {}
            for mt in range(32):
                mt_off[mt] = off
                for (kt, m2) in st["big"]:
                    if m2 == mt:
                        by_mt[mt].append(("big", off, kt, 0))
                        off += 1
            for (kt, mt, w, col) in st["packed"]:
                by_mt[mt].append(("small", col, kt, w))

            def cp(i, dst, src):
                # PSUM->SBUF copies alternate Act/DVE (Pool is slow)
                if i % 2 == 0:
                    nc.scalar.copy(dst, src)
                else:
                    nc.vector.tensor_copy(dst, src)

            def veng(i):
                return nc.gpsimd if i % 4 == 3 else nc.vector

            rep_ctx = tc.For_i(0, reps, 1) if reps > 1 else None
            if rep_ctx is not None:
                rep_ctx.__enter__()
            for ci_ in range(N_CHUNKS):
                r0 = ci_ * CHUNK
                # ---- transpose-in
                xT = stagep.tile([128, FTILES * CHUNK], _BF16, tag="stage")
                nt0 = r0 // 128
                for ntp in range(2):
                    xbfs = []
                    for nn in range(2):
                        xbf = xbfp.tile([128, D_FEAT], _BF16, tag="xbf")
                        nc.gpsimd.dma_start(
                            xbf[:], xs[:, nt0 + ntp * 2 + nn, :])
                        xbfs.append(xbf)
                    for fc in range(FTILES):
                        ps = tpp.tile([128, 512], _BF16, tag="tp")
                        for nn in range(2):
                            nc.tensor.transpose(
                                ps[:, nn * 128:(nn + 1) * 128],
                                xbfs[nn][:, fc * 128:(fc + 1) * 128], ident[:])
                        eng = nc.vector if fc % 2 else nc.scalar
                        (eng.tensor_copy if eng is nc.vector else eng.copy)(
                            xT[:, fc * CHUNK + ntp * 256:
                               fc * CHUNK + ntp * 256 + 256],
                            ps[:, 0:256])

                # ---- z1 = gconvA with orientation-folded lhsT
                z1 = stagep.tile([128, FTILES * CHUNK], _BF16, tag="stage")
                for t in range(FTILES):
                    ps = mmp.tile([128, CHUNK], _F32, tag="mm")
                    nc.tensor.matmul(
                        ps[:], atv_t[:, t, :],
                        xT[:, t * CHUNK:(t + 1) * CHUNK],
                        start=True, stop=True)
                    cp(t, z1[:, t * CHUNK:(t + 1) * CHUNK], ps[:])

                def sl(arr, t):
                    return arr[:, t * CHUNK:(t + 1) * CHUNK]

                # ---- fused L1+L2 stages (a1/a2/b1/b2 = L1 partials)
                s2 = stagep.tile([128, FTILES * CHUNK], _BF16, tag="stage")
                for t in range(8):
                    a1 = tmpp.tile([128, CHUNK], _BF16, tag="tmp")
                    a2 = tmpp.tile([128, CHUNK], _BF16, tag="tmp")
                    b1 = tmpp.tile([128, CHUNK], _BF16, tag="tmp")
                    b2 = tmpp.tile([128, CHUNK], _BF16, tag="tmp")
                    nc.vector.tensor_add(a1[:], sl(z1, t), sl(z1, 31 - t))
                    nc.gpsimd.tensor_add(a2[:], sl(z1, 15 - t), sl(z1, 16 + t))
                    nc.vector.tensor_sub(b1[:], sl(z1, t), sl(z1, 31 - t))
                    nc.gpsimd.tensor_sub(b2[:], sl(z1, 15 - t), sl(z1, 16 + t))
                    nc.vector.tensor_add(sl(s2, t), a1[:], a2[:])
                    nc.vector.tensor_sub(sl(s2, 8 + t), a1[:], a2[:])
                    # p = c*b1 + s*b2 ; qt = qa*b1 + qb*b2
                    tm = tmpp.tile([128, CHUNK], _BF16, tag="tmp")
                    nc.gpsimd.tensor_scalar(
                        tm[:], b1[:], cf_t[:, 0, t:t + 1], None, _MUL)
                    nc.vector.scalar_tensor_tensor(
                        sl(s2, 16 + t), b2[:], cf_t[:, 1, t:t + 1],
                        tm[:], _MUL, _ADD)
                    tm2 = tmpp.tile([128, CHUNK], _BF16, tag="tmp")
                    nc.gpsimd.tensor_scalar(
                        tm2[:], b1[:], cf_t[:, 2, t:t + 1], None, _MUL)
                    nc.vector.scalar_tensor_tensor(
                        sl(s2, 24 + t), b2[:], cf_t[:, 3, t:t + 1],
                        tm2[:], _MUL, _ADD)

                # ---- fwd leaves: z2leaf[8L+m] = sum_tau W[L,m,tau] @ s2[8L+tau]
                z2 = stagep.tile([128, FTILES * CHUNK], _BF16, tag="stage")
                for L in range(4):
                    for m in range(8):
                        fw = fwp.tile([128, 8, 128], _BF16, tag="fw")
                        nc.sync.dma_start(fw[:], fwdw[L, m])
                        ps = mmp.tile([128, CHUNK], _F32, tag="mm")
                        for tau in range(8):
                            nc.tensor.matmul(
                                ps[:], fw[:, tau, :], sl(s2, 8 * L + tau),
                                start=(tau == 0), stop=(tau == 7))
                        cp(m, sl(z2, 8 * L + m), ps[:])

                # ---- middle: z3leaf[mt] = sum_kt Meff[kt,mt]^T-app @ z2[kt]
                z3 = stagep.tile([128, FTILES * CHUNK], _BF16, tag="stage")
                for mt in range(32):
                    lst = by_mt[mt]
                    nb = sum(1 for e in lst if e[0] == "big")
                    mtile = mfp.tile([128, max(nb, 1), 128], _BF16, tag="mf")
                    if nb:
                        nc.sync.dma_start(
                            mtile[:, 0:nb, :],
                            meffw[:, mt_off[mt]:mt_off[mt] + nb, :])
                    ps = mmp.tile([128, CHUNK], _F32, tag="mm")
                    for li, (kind, bi, kt, w) in enumerate(lst):
                        if kind == "big":
                            lhsT = mtile[:, bi - mt_off[mt], :]
                            rhs = sl(z2, kt)
                        else:
                            b0 = 64 if w == 3 else 32 * w
                            kk = 64 if w == 3 else 32
                            lhsT = small_t[b0:b0 + kk, bi, :]
                            rhs = z2[b0:b0 + kk,
                                     kt * CHUNK:(kt + 1) * CHUNK]
                        nc.tensor.matmul(
                            ps[:], lhsT, rhs,
                            start=(li == 0), stop=(li == len(lst) - 1))
                    cp(mt, sl(z3, mt), ps[:])

                # ---- inverse: leaves (data-stationary) + stages + stores
                for strip in range(NSTRIP):
                    ivw = []
                    for wave in range(2):
                        iv = ivp.tile([128, 2, 8, QW], _BF16, tag="iv")
                        nc.sync.dma_start(iv[:], invw[strip, wave])
                        ivw.append(iv)
                    for nt in range(CHUNK // 128):
                        sb = []   # stage-space tiles ulo,uhi,p,qt in SBUF bf16
                        for wave in range(2):
                            pss = []
                            for b in range(2):
                                L = 2 * wave + b
                                ps = stp.tile([128, QW], _F32, tag="st")
                                for kt in range(8):
                                    nc.tensor.matmul(
                                        ps[:],
                                        z3[:, (8 * L + kt) * CHUNK + nt * 128:
                                           (8 * L + kt) * CHUNK + (nt + 1) * 128],
                                        ivw[wave][:, b, kt, :],
                                        start=(kt == 0), stop=False)
                                nc.tensor.matmul(
                                    ps[:], ones1[:], bst_t[0:1, L, strip, :],
                                    start=False, stop=True)
                                pss.append(ps)
                            for b in range(2):
                                t = ostp.tile([128, QW], _BF16, tag="ost")
                                cp(wave * 2 + b, t[:], pss[b][:])
                                sb.append(t)
                        ulo, uhi, pp, qq = sb
                        u_lo = tmpp.tile([128, QW], _BF16, tag="tmp")
                        u_hi = tmpp.tile([128, QW], _BF16, tag="tmp")
                        nc.vector.tensor_add(u_lo[:], ulo[:], uhi[:])
                        nc.vector.tensor_sub(u_hi[:], ulo[:], uhi[:])
                        m1 = tmpp.tile([128, QW], _BF16, tag="tmp")
                        m2 = tmpp.tile([128, QW], _BF16, tag="tmp")
                        v_lo = tmpp.tile([128, QW], _BF16, tag="tmp")
                        v_hi = tmpp.tile([128, QW], _BF16, tag="tmp")
                        # v_lo = c*p + qa*qt ; v_hi = s*p + qb*qt
                        nc.vector.tensor_mul(m1[:], pp[:], ci_t[:, 0, strip, :])
                        nc.gpsimd.tensor_mul(m2[:], qq[:], ci_t[:, 2, strip, :])
                        nc.vector.tensor_add(v_lo[:], m1[:], m2[:])
                        m3 = tmpp.tile([128, QW], _BF16, tag="tmp")
                        m4 = tmpp.tile([128, QW], _BF16, tag="tmp")
                        nc.gpsimd.tensor_mul(m3[:], pp[:], ci_t[:, 1, strip, :])
                        nc.vector.tensor_mul(m4[:], qq[:], ci_t[:, 3, strip, :])
                        nc.vector.tensor_add(v_hi[:], m3[:], m4[:])
                        rows = slice(r0 + nt * 128, r0 + (nt + 1) * 128)
                        combos = [(u_lo, v_lo, 1), (u_lo, v_lo, -1),
                                  (u_hi, v_hi, 1), (u_hi, v_hi, -1)]
                        for si, (ua, va, sg) in enumerate(combos):
                            start, rev = RUN[strip][si]
                            ot = ostp.tile([128, QW], _BF16, tag="ost")
                            dst = ot[:, ::-1] if rev else ot[:]
                            if sg > 0:
                                veng(si).tensor_add(dst, ua[:], va[:])
                            else:
                                veng(si).tensor_sub(dst, ua[:], va[:])
                            nc.gpsimd.dma_start(
                                out[rows, start:start + QW], ot[:])
            if rep_ctx is not None:
                rep_ctx.__exit__(None, None, None)
    nc.finalize()
    return nc


_CACHE = {}


def _prep():
    if "static" not in _CACHE:
        _CACHE["static"] = _host_static()
    return _CACHE["static"]


def kernel(x, A, D, bias):
    st = _prep()
    if "nc" not in _CACHE:
        _CACHE["nc"] = _build_program()
    nc = _CACHE["nc"]

    # gconvA lhsT with orientation folding: slot t rev for 8-15, 24-31
    At = np.asarray(A, dtype=np.float32)
    atv = np.zeros((128, FTILES, 128), np.float32)
    for t in range(FTILES):
        W = At[t].T                       # lhsT [i, o]
        if (8 <= t < 16) or (24 <= t < 32):
            W = W[:, ::-1]
        atv[:, t, :] = W
    atv = atv.astype(ml_dtypes.bfloat16)

    Meff = _build_meff(np.asarray(D, dtype=np.float64), *st["lift"])
    meffw = np.zeros((128, len(st["big"]), 128), np.float32)
    off = 0
    for mt in range(32):     # sorted by mt to match streaming order
        for (kt, m2) in st["big"]:
            if m2 == mt:
                meffw[:, off, :] = Meff[kt * 128:(kt + 1) * 128,
                                        mt * 128:(mt + 1) * 128]
                off += 1
    meffw = meffw.astype(ml_dtypes.bfloat16)
    smallw = np.zeros((128, st["ncol"], 128), np.float32)
    for (kt, mt, w, col) in st["packed"]:
        smallw[32 * w:32 * w + 32, col, :] = \
            Meff[kt * 128 + 32 * w:kt * 128 + 32 * w + 32,
                 mt * 128:(mt + 1) * 128]
    smallw = smallw.astype(ml_dtypes.bfloat16)

    # bias pre-image in stage space
    rc, rs, qa, qb = st["rot"]
    bias_v = np.asarray(bias, dtype=np.float64).reshape(-1)
    j = np.arange(Q)
    # riffle position of feature index f: pos = (f%2)*2048 + f//2
    def rif(f):
        return (f % 2) * 2048 + f // 2
    t1 = bias_v[rif(j)]
    t2 = bias_v[rif(4095 - j)]
    t3 = bias_v[rif(2047 - j)]
    t4 = bias_v[rif(2048 + j)]
    U1, V1 = (t1 + t2) / 2, (t1 - t2) / 2
    U2, V2 = (t3 + t4) / 2, (t3 - t4) / 2
    b_ulo, b_uhi = (U1 + U2) / 2, (U1 - U2) / 2
    b_p = rc * V1 + rs * V2
    b_qt = qa * V1 + qb * V2
    bst = np.stack([b_ulo, b_uhi, b_p, b_qt])          # [4, 1024]
    bst = bst[:, st["colperm"]].reshape(4, 2, 512)[None]
    bst = bst.astype(ml_dtypes.bfloat16)

    x = np.ascontiguousarray(x, dtype=np.float32)
    in_maps = []
    for c in range(N_CORES):
        shard = x[c * N_SHARD:(c + 1) * N_SHARD]
        xs_t = np.ascontiguousarray(
            shard.reshape(N_SHARD // 128, 128, D_FEAT).transpose(1, 0, 2))
        in_maps.append({
            "xs": xs_t, "atv": atv, "fwdw": st["fwdw"], "meffw": meffw,
            "smallw": smallw, "invw": st["invw"], "cfw": st["cfw"],
            "ciw": st["ciw"], "bstw": bst,
        })
    res = run_bass_kernel_spmd(nc, in_maps, core_ids=list(range(N_CORES)))
    return np.concatenate([res.results[c]["out"] for c in range(N_CORES)],
                          axis=0)


# revision 82
# speedup vs baseline: 1.2219x; 1.1961x over previous
"""Trainium2 Bass kernel for BlockDiagonalACDC — depth-2 DCT factorization.

out = riffle(idct2(gconv(dct2(gconv(x, A)), D))) + bias.

The 4096-point DCT-II is factored two levels deep:
  L1 fold:  u = f[:2048]+rev(f[2048:]), v = f[:2048]-rev(f[2048:])
  L2 fold:  ulo/uhi from u;  Givens rotations (p, qt) from v
  leaves:   four dense 1024x1024 matmuls (DCT2/DCT4/DCT2/DCT2)
  lift:     odd-k outputs are +-1 pairs of leaf2/leaf3 outputs, folded with
            gconv(D) and the idct input diag into one middle matrix Meff
            (applied as ~140 sparse 128x128 blocks).
  inverse:  transposed leaves (data-stationary), then transposed rotations /
            folds on [batch, j] tiles with broadcast coeff rows; riffle and
            the final reversal land as 8 contiguous 512-wide output runs.

z1 orientation trick: gconvA lhsT column-reversed for tiles 8-15/24-31 so all
fold/rotation pairings are partition-aligned elementwise DVE ops.

Sharding: pure data parallel, 2048 batch rows per core on 8 cores.
"""

import numpy as np
import ml_dtypes

import concourse.bacc as bacc
import concourse.mybir as mybir
from concourse.tile import TileContext
from concourse.bass_utils import run_bass_kernel_spmd
from concourse.masks import make_identity

N_BATCH, D_FEAT, GROUPS = 16384, 4096, 32
N_CORES = 8
N_SHARD = N_BATCH // N_CORES      # 2048 rows per core
CHUNK = 512
N_CHUNKS = N_SHARD // CHUNK       # 4
FTILES = D_FEAT // 128            # 32
N, H, Q = 4096, 2048, 1024

_BF16 = mybir.dt.bfloat16
_F32 = mybir.dt.float32
_MUL = mybir.AluOpType.mult
_ADD = mybir.AluOpType.add

# inverse output runs (512 wide): [strip][slot fa..fd] -> (start, reversed)
# strip 0 = even j' (asc), strip 1 = odd j' (colperm order)
RUN = [[(0, False), (3584, True), (2560, True), (1024, False)],
       [(2048, False), (1536, True), (512, True), (3072, False)]]
QW = 512
NSTRIP = 2


def _dct2m(n):
    k = np.arange(n)[:, None]; j = np.arange(n)[None, :]
    return np.cos(np.pi * k * (2 * j + 1) / (2 * n))


def _dct4m(n):
    m = np.arange(n)[:, None]; j = np.arange(n)[None, :]
    return np.cos(np.pi * (2 * m + 1) * (2 * j + 1) / (4 * n))


def _dst_sign():
    h = 8
    r = np.arange(h)[:, None]; j = np.arange(h)[None, :]
    d2 = np.cos(np.pi * r * (2 * j + 1) / (2 * h))
    s2 = np.sin(np.pi * (r + 1) * (2 * j + 1) / (2 * h))
    q = np.random.RandomState(0).randn(h)
    qt = q * np.where(np.arange(h) % 2 == 0, 1.0, -1.0)
    b = s2 @ q
    return 1.0 if np.allclose(b, (d2 @ qt)[::-1]) else -1.0


def _host_static():
    """A/D/bias-independent constants."""
    dst_sign = _dst_sign()
    # leaf3 col r holds B[r-1] = dst_sign*2*dct2(qt)[1024-r] (col 0: B[1023])
    # so lift pairs (leaf2 col r, leaf3 col r) are tile-aligned
    l3rows = np.r_[0, np.arange(1023, 0, -1)]
    leaf = [2.0 * _dct2m(Q), 2.0 * _dct4m(Q), 2.0 * _dct2m(Q),
            2.0 * _dct2m(Q)[l3rows]]

    alpha = (2 * np.arange(Q) + 1) * np.pi / (4 * H)
    rc, rs = np.cos(alpha), np.sin(alpha)
    sgn = np.where(np.arange(Q) % 2 == 0, 1.0, -1.0)
    qa, qb = -sgn * rs, sgn * rc

    # lift table: k -> up to 2 (leaf-flat index a, sign)
    A1 = np.zeros(N, np.int64); S1 = np.zeros(N)
    A2 = np.zeros(N, np.int64); S2 = np.zeros(N)
    for r in range(Q):
        A1[4 * r], S1[4 * r] = 0 * Q + r, 1.0
        A1[4 * r + 2], S1[4 * r + 2] = 1 * Q + r, 1.0
    A1[1], S1[1] = 2 * Q + 0, 1.0
    for r in range(1, Q):
        # leaf3 col r = B[r-1] (dst_sign folded into lift signs below)
        A1[4 * r - 1], S1[4 * r - 1] = 2 * Q + r, 1.0
        A2[4 * r - 1], S2[4 * r - 1] = 3 * Q + r, -dst_sign
        A1[4 * r + 1], S1[4 * r + 1] = 2 * Q + r, 1.0
        A2[4 * r + 1], S2[4 * r + 1] = 3 * Q + r, dst_sign
    A1[4 * Q - 1], S1[4 * Q - 1] = 3 * Q + 0, -dst_sign

    # fwd leaf weights: fwdw[L, m, i, tau, c] = leaf[L][128m+c, 128tau+i]
    fwdw = np.stack([
        M.T.reshape(8, 128, 8, 128).transpose(2, 1, 0, 3) for M in leaf
    ]).astype(ml_dtypes.bfloat16)

    # inverse: stage-col parity permutation, per strip
    colperm = np.concatenate([np.arange(0, Q, 2), np.arange(1, Q, 2)])
    # invw[strip, wave, i, b, kt, c] =
    #     (leaf[2*wave+b]/2)[128kt+i, colperm[512*strip+c]]
    pieces = [(M / 2.0)[:, colperm].reshape(8, 128, 2, 512) for M in leaf]
    invw = np.zeros((2, 2, 128, 2, 8, 512), np.float32)
    for L in range(4):
        # pieces[L][kt, i, s, c] -> invw[s, L//2, i, L%2, kt, c]
        invw[:, L // 2, :, L % 2, :, :] = pieces[L].transpose(2, 1, 0, 3)
    invw = invw.astype(ml_dtypes.bfloat16)

    # fwd rotation per-partition coeffs cfw[p, cf, t'] (cf: c,s,qa,qb)
    cfw = np.stack([rc, rs, qa, qb]).reshape(4, 8, 128).transpose(2, 0, 1)
    cfw = np.ascontiguousarray(cfw).astype(np.float32)

    # inverse rotation free-dim coeffs ciw[p, cf, strip, c] (broadcast rows)
    ci = np.stack([rc, rs, qa, qb])[:, colperm].reshape(4, 2, 512)
    ciw = np.broadcast_to(ci, (128, 4, 2, 512)).astype(ml_dtypes.bfloat16)
    ciw = np.ascontiguousarray(ciw)

    # Meff block sparsity pattern (structure only): big blocks are K=128
    # matmuls; sparse spill blocks decompose into <=32-row pieces (K=32).
    rng = np.random.RandomState(0)
    Mp = _build_meff(rng.randn(GROUPS, 128, 128), A1, S1, A2, S2)
    big = []     # (kt, mt)
    small = []   # (kt, mt, w): 32-row window w (w=3 runs as K=64 from base 64)
    for kt in range(32):
        for mt in range(32):
            blk = Mp[kt * 128:(kt + 1) * 128, mt * 128:(mt + 1) * 128]
            rows = np.where(np.abs(blk).max(axis=1) > 1e-12)[0]
            if len(rows) == 0:
                continue
            if len(rows) > 48:
                big.append((kt, mt))
                continue
            for w in sorted(set(int(r) // 32 for r in rows)):
                small.append((kt, mt, w))
    # pack small pieces into tile columns by window class; window 3 runs as
    # K=64 from base 64 so it must not share a column with a window-2 piece.
    cls = {w: [p for p in small if p[2] == w] for w in range(4)}
    ncol = max(len(cls[0]), len(cls[1]), len(cls[2]) + len(cls[3]), 1)
    packed = []  # (kt, mt, w, col)
    for w in (0, 1):
        for j, (kt, mt, _) in enumerate(cls[w]):
            packed.append((kt, mt, w, j))
    for j, (kt, mt, _) in enumerate(cls[2]):
        packed.append((kt, mt, 2, j))
    for j, (kt, mt, _) in enumerate(cls[3]):
        packed.append((kt, mt, 3, len(cls[2]) + j))

    return dict(leaf=leaf, fwdw=fwdw, invw=invw, cfw=cfw, ciw=ciw,
                colperm=colperm, lift=(A1, S1, A2, S2), big=big,
                packed=packed, ncol=ncol, rot=(rc, rs, qa, qb))


def _build_meff(D, A1, S1, A2, S2):
    w = np.ones(N); w[0] = 0.5
    wN = w / N
    M = np.zeros((N, N))
    for g in range(GROUPS):
        kk = np.arange(128) + 128 * g
        blk = D[g].T * wN[kk][None, :]        # [i, o]
        for (ai, si) in ((A1[kk], S1[kk]), (A2[kk], S2[kk])):
            for (ao, so) in ((A1[kk], S1[kk]), (A2[kk], S2[kk])):
                contrib = (si[:, None] * so[None, :]) * blk
                np.add.at(M, (ai[:, None], ao[None, :]), contrib)
    return M


def _build_program(reps=1):
    st = _prep()
    nbig = len(st["big"])
    nsmt = st["ncol"]
    nc = bacc.Bacc()
    # host-pre-transposed input: xs[ftile, part, batchrow]
    xs = nc.dram_tensor("xs", (FTILES, 128, N_SHARD), _F32,
                        kind="ExternalInput")
    # 32 base lhsT tiles + negated copies of tiles 16..31 (slots 32..47)
    atv = nc.dram_tensor("atv", (128, FTILES + 16, 128), _BF16,
                         kind="ExternalInput")
    fwdw = nc.dram_tensor("fwdw", (4, 8, 128, 8, 128), _BF16,
                          kind="ExternalInput")
    meffw = nc.dram_tensor("meffw", (128, nbig, 128), _BF16,
                           kind="ExternalInput")
    smallw = nc.dram_tensor("smallw", (128, max(nsmt, 1), 128), _BF16,
                            kind="ExternalInput")
    invw = nc.dram_tensor("invw", (2, 2, 128, 2, 8, 512), _BF16,
                          kind="ExternalInput")
    cfw = nc.dram_tensor("cfw", (128, 4, 8), _F32, kind="ExternalInput")
    ciw = nc.dram_tensor("ciw", (128, 4, 2, 512), _BF16, kind="ExternalInput")
    bstw = nc.dram_tensor("bstw", (1, 4, 2, 512), _BF16, kind="ExternalInput")
    out = nc.dram_tensor("out", (N_SHARD, D_FEAT), _F32, kind="ExternalOutput")

    with TileContext(nc) as tc:
        with (
            tc.tile_pool(name="const", bufs=1) as constp,
            tc.tile_pool(name="stage", bufs=3) as stagep,
            tc.tile_pool(name="fwp", bufs=4) as fwp,
            tc.tile_pool(name="mfp", bufs=3) as mfp,
            tc.tile_pool(name="ivp", bufs=2) as ivp,
            tc.tile_pool(name="tmp", bufs=8) as tmpp,
            tc.tile_pool(name="ost", bufs=6) as ostp,
            tc.tile_pool(name="mm_ps", bufs=4, space="PSUM") as mmp,
            tc.tile_pool(name="st_ps", bufs=4, space="PSUM") as stp,
        ):
            ident = constp.tile([128, 128], _BF16, tag="ident")
            make_identity(nc, ident[:])
            ones1 = constp.tile([1, 128], _BF16, tag="ones1")
            nc.gpsimd.memset(ones1[:], 1.0)
            atv_t = constp.tile([128, FTILES + 16, 128], _BF16, tag="atv")
            nc.scalar.dma_start(atv_t[:], atv[:])
            small_t = constp.tile([128, max(nsmt, 1), 128], _BF16, tag="smeff")
            nc.scalar.dma_start(small_t[:], smallw[:])
            cf_t = constp.tile([128, 4, 8], _F32, tag="cf")
            nc.scalar.dma_start(cf_t[:], cfw[:])
            ci_t = constp.tile([128, 4, 2, 512], _BF16, tag="ci")
            nc.scalar.dma_start(ci_t[:], ciw[:])
            bst_t = constp.tile([1, 4, 2, 512], _BF16, tag="bst")
            nc.scalar.dma_start(bst_t[:], bstw[:])

            # middle-matmul emission lists per output tile; big blocks are
            # stored in meffw sorted by mt (kernel() packs them identically)
            by_mt = {mt: [] for mt in range(32)}
            off = 0
            mt_off = {}
            for mt in range(32):
                mt_off[mt] = off
                for (kt, m2) in st["big"]:
                    if m2 == mt:
                        by_mt[mt].append(("big", off, kt, 0))
                        off += 1
            for (kt, mt, w, col) in st["packed"]:
                by_mt[mt].append(("small", col, kt, w))

            def cp(i, dst, src):
                # PSUM->SBUF copies alternate Act/DVE (Pool is slow)
                if i % 2 == 0:
                    nc.scalar.copy(dst, src)
                else:
                    nc.vector.tensor_copy(dst, src)

            def veng(i):
                return nc.gpsimd if i % 4 == 3 else nc.vector

            rep_ctx = tc.For_i(0, reps, 1) if reps > 1 else None
            if rep_ctx is not None:
                rep_ctx.__enter__()
            for ci_ in range(N_CHUNKS):
                r0 = ci_ * CHUNK
                # ---- load host-pre-transposed input (f32->bf16 cast DMA)
                xT = stagep.tile([128, FTILES * CHUNK], _BF16, tag="stage")
                for t in range(FTILES):
                    nc.gpsimd.dma_start(
                        xT[:, t * CHUNK:(t + 1) * CHUNK],
                        xs[t, :, r0:r0 + CHUNK])

                def sl(arr, t):
                    return arr[:, t * CHUNK:(t + 1) * CHUNK]

                def xsl(t):
                    return xT[:, t * CHUNK:(t + 1) * CHUNK]

                # ---- L1 partials on PE (2-term accumulated gconvA matmuls):
                # a1 = z1[t]+z1[31-t], a2 = z1[15-t]+z1[16+t],
                # b1 = z1[t]-z1[31-t], b2 = z1[15-t]-z1[16+t]
                # then L2 folds/rotations on DVE into s2.
                s2 = stagep.tile([128, FTILES * CHUNK], _BF16, tag="stage")
                for t in range(8):
                    # (lhs slot, rhs xT tile) pairs; slots 32..47 hold
                    # -atv[16..31]
                    combos2 = (
                        ((t, t), (31 - t, 31 - t)),             # a1
                        ((15 - t, 15 - t), (16 + t, 16 + t)),   # a2
                        ((t, t), (47 - t, 31 - t)),             # b1
                        ((15 - t, 15 - t), (32 + t, 16 + t)),   # b2
                    )
                    parts = []
                    for pi, pair in enumerate(combos2):
                        ps = mmp.tile([128, CHUNK], _F32, tag="mm")
                        for mi, (lslot, rtile) in enumerate(pair):
                            nc.tensor.matmul(
                                ps[:], atv_t[:, lslot, :], xsl(rtile),
                                start=(mi == 0), stop=(mi == 1))
                        pt = tmpp.tile([128, CHUNK], _BF16, tag="tmp")
                        cp(pi, pt[:], ps[:])
                        parts.append(pt)
                    a1, a2, b1, b2 = parts
                    nc.vector.tensor_add(sl(s2, t), a1[:], a2[:])
                    nc.vector.tensor_sub(sl(s2, 8 + t), a1[:], a2[:])
                    # p = c*b1 + s*b2 ; qt = qa*b1 + qb*b2
                    tm = tmpp.tile([128, CHUNK], _BF16, tag="tmp")
                    nc.gpsimd.tensor_scalar(
                        tm[:], b1[:], cf_t[:, 0, t:t + 1], None, _MUL)
                    nc.vector.scalar_tensor_tensor(
                        sl(s2, 16 + t), b2[:], cf_t[:, 1, t:t + 1],
                        tm[:], _MUL, _ADD)
                    tm2 = tmpp.tile([128, CHUNK], _BF16, tag="tmp")
                    nc.gpsimd.tensor_scalar(
                        tm2[:], b1[:], cf_t[:, 2, t:t + 1], None, _MUL)
                    nc.vector.scalar_tensor_tensor(
                        sl(s2, 24 + t), b2[:], cf_t[:, 3, t:t + 1],
                        tm2[:], _MUL, _ADD)

                # ---- fwd leaves: z2leaf[8L+m] = sum_tau W[L,m,tau] @ s2[8L+tau]
                z2 = stagep.tile([128, FTILES * CHUNK], _BF16, tag="stage")
                for L in range(4):
                    for m in range(8):
                        fw = fwp.tile([128, 8, 128], _BF16, tag="fw")
                        nc.sync.dma_start(fw[:], fwdw[L, m])
                        ps = mmp.tile([128, CHUNK], _F32, tag="mm")
                        for tau in range(8):
                            nc.tensor.matmul(
                                ps[:], fw[:, tau, :], sl(s2, 8 * L + tau),
                                start=(tau == 0), stop=(tau == 7))
                        cp(m, sl(z2, 8 * L + m), ps[:])

                # ---- middle: z3leaf[mt] = sum_kt Meff[kt,mt]^T-app @ z2[kt]
                z3 = stagep.tile([128, FTILES * CHUNK], _BF16, tag="stage")
                for mt in range(32):
                    lst = by_mt[mt]
                    nb = sum(1 for e in lst if e[0] == "big")
                    mtile = mfp.tile([128, max(nb, 1), 128], _BF16, tag="mf")
                    if nb:
                        nc.sync.dma_start(
                            mtile[:, 0:nb, :],
                            meffw[:, mt_off[mt]:mt_off[mt] + nb, :])
                    ps = mmp.tile([128, CHUNK], _F32, tag="mm")
                    for li, (kind, bi, kt, w) in enumerate(lst):
                        if kind == "big":
                            lhsT = mtile[:, bi - mt_off[mt], :]
                            rhs = sl(z2, kt)
                        else:
                            b0 = 64 if w == 3 else 32 * w
                            kk = 64 if w == 3 else 32
                            lhsT = small_t[b0:b0 + kk, bi, :]
                            rhs = z2[b0:b0 + kk,
                                     kt * CHUNK:(kt + 1) * CHUNK]
                        nc.tensor.matmul(
                            ps[:], lhsT, rhs,
                            start=(li == 0), stop=(li == len(lst) - 1))
                    cp(mt, sl(z3, mt), ps[:])

                # ---- inverse: leaves (data-stationary) + stages + stores
                for strip in range(NSTRIP):
                    ivw = []
                    for wave in range(2):
                        iv = ivp.tile([128, 2, 8, QW], _BF16, tag="iv")
                        nc.sync.dma_start(iv[:], invw[strip, wave])
                        ivw.append(iv)
                    for nt in range(CHUNK // 128):
                        sb = []   # stage-space tiles ulo,uhi,p,qt in SBUF bf16
                        for wave in range(2):
                            pss = []
                            for b in range(2):
                                L = 2 * wave + b
                                ps = stp.tile([128, QW], _F32, tag="st")
                                for kt in range(8):
                                    nc.tensor.matmul(
                                        ps[:],
                                        z3[:, (8 * L + kt) * CHUNK + nt * 128:
                                           (8 * L + kt) * CHUNK + (nt + 1) * 128],
                                        ivw[wave][:, b, kt, :],
                                        start=(kt == 0), stop=False)
                                nc.tensor.matmul(
                                    ps[:], ones1[:], bst_t[0:1, L, strip, :],
                                    start=False, stop=True)
                                pss.append(ps)
                            for b in range(2):
                                t = ostp.tile([128, QW], _BF16, tag="ost")
                                cp(wave * 2 + b, t[:], pss[b][:])
                                sb.append(t)
                        ulo, uhi, pp, qq = sb
                        u_lo = tmpp.tile([128, QW], _BF16, tag="tmp")
                        u_hi = tmpp.tile([128, QW], _BF16, tag="tmp")
                        nc.vector.tensor_add(u_lo[:], ulo[:], uhi[:])
                        nc.vector.tensor_sub(u_hi[:], ulo[:], uhi[:])
                        m1 = tmpp.tile([128, QW], _BF16, tag="tmp")
                        m2 = tmpp.tile([128, QW], _BF16, tag="tmp")
                        v_lo = tmpp.tile([128, QW], _BF16, tag="tmp")
                        v_hi = tmpp.tile([128, QW], _BF16, tag="tmp")
                        # v_lo = c*p + qa*qt ; v_hi = s*p + qb*qt
                        nc.vector.tensor_mul(m1[:], pp[:], ci_t[:, 0, strip, :])
                        nc.gpsimd.tensor_mul(m2[:], qq[:], ci_t[:, 2, strip, :])
                        nc.vector.tensor_add(v_lo[:], m1[:], m2[:])
                        m3 = tmpp.tile([128, QW], _BF16, tag="tmp")
                        m4 = tmpp.tile([128, QW], _BF16, tag="tmp")
                        nc.gpsimd.tensor_mul(m3[:], pp[:], ci_t[:, 1, strip, :])
                        nc.vector.tensor_mul(m4[:], qq[:], ci_t[:, 3, strip, :])
                        nc.vector.tensor_add(v_hi[:], m3[:], m4[:])
                        rows = slice(r0 + nt * 128, r0 + (nt + 1) * 128)
                        combos = [(u_lo, v_lo, 1), (u_lo, v_lo, -1),
                                  (u_hi, v_hi, 1), (u_hi, v_hi, -1)]
                        for si, (ua, va, sg) in enumerate(combos):
                            start, rev = RUN[strip][si]
                            ot = ostp.tile([128, QW], _BF16, tag="ost")
                            dst = ot[:, ::-1] if rev else ot[:]
                            if sg > 0:
                                veng(si).tensor_add(dst, ua[:], va[:])
                            else:
                                veng(si).tensor_sub(dst, ua[:], va[:])
                            nc.gpsimd.dma_start(
                                out[rows, start:start + QW], ot[:])
            if rep_ctx is not None:
                rep_ctx.__exit__(None, None, None)
    nc.finalize()
    return nc


_CACHE = {}


def _prep():
    if "static" not in _CACHE:
        _CACHE["static"] = _host_static()
    return _CACHE["static"]


def kernel(x, A, D, bias):
    st = _prep()
    if "nc" not in _CACHE:
        _CACHE["nc"] = _build_program()
    nc = _CACHE["nc"]

    # gconvA lhsT with orientation folding: slot t rev for 8-15, 24-31;
    # slots 32..47 = negated copies of slots 16..31 (for b1/b2 partials)
    At = np.asarray(A, dtype=np.float32)
    atv = np.zeros((128, FTILES + 16, 128), np.float32)
    for t in range(FTILES):
        W = At[t].T                       # lhsT [i, o]
        if (8 <= t < 16) or (24 <= t < 32):
            W = W[:, ::-1]
        atv[:, t, :] = W
    atv[:, 32:48, :] = -atv[:, 16:32, :]
    atv = atv.astype(ml_dtypes.bfloat16)

    Meff = _build_meff(np.asarray(D, dtype=np.float64), *st["lift"])
    meffw = np.zeros((128, len(st["big"]), 128), np.float32)
    off = 0
    for mt in range(32):     # sorted by mt to match streaming order
        for (kt, m2) in st["big"]:
            if m2 == mt:
                meffw[:, off, :] = Meff[kt * 128:(kt + 1) * 128,
                                        mt * 128:(mt + 1) * 128]
                off += 1
    meffw = meffw.astype(ml_dtypes.bfloat16)
    smallw = np.zeros((128, st["ncol"], 128), np.float32)
    for (kt, mt, w, col) in st["packed"]:
        smallw[32 * w:32 * w + 32, col, :] = \
            Meff[kt * 128 + 32 * w:kt * 128 + 32 * w + 32,
                 mt * 128:(mt + 1) * 128]
    smallw = smallw.astype(ml_dtypes.bfloat16)

    # bias pre-image in stage space
    rc, rs, qa, qb = st["rot"]
    bias_v = np.asarray(bias, dtype=np.float64).reshape(-1)
    j = np.arange(Q)
    # riffle position of feature index f: pos = (f%2)*2048 + f//2
    def rif(f):
        return (f % 2) * 2048 + f // 2
    t1 = bias_v[rif(j)]
    t2 = bias_v[rif(4095 - j)]
    t3 = bias_v[rif(2047 - j)]
    t4 = bias_v[rif(2048 + j)]
    U1, V1 = (t1 + t2) / 2, (t1 - t2) / 2
    U2, V2 = (t3 + t4) / 2, (t3 - t4) / 2
    b_ulo, b_uhi = (U1 + U2) / 2, (U1 - U2) / 2
    b_p = rc * V1 + rs * V2
    b_qt = qa * V1 + qb * V2
    bst = np.stack([b_ulo, b_uhi, b_p, b_qt])          # [4, 1024]
    bst = bst[:, st["colperm"]].reshape(4, 2, 512)[None]
    bst = bst.astype(ml_dtypes.bfloat16)

    x = np.ascontiguousarray(x, dtype=np.float32)
    in_maps = []
    for c in range(N_CORES):
        shard = x[c * N_SHARD:(c + 1) * N_SHARD]
        xs_t = np.ascontiguousarray(shard.T).reshape(FTILES, 128, N_SHARD)
        in_maps.append({
            "xs": xs_t, "atv": atv, "fwdw": st["fwdw"], "meffw": meffw,
            "smallw": smallw, "invw": st["invw"], "cfw": st["cfw"],
            "ciw": st["ciw"], "bstw": bst,
        })
    res = run_bass_kernel_spmd(nc, in_maps, core_ids=list(range(N_CORES)))
    return np.concatenate([res.results[c]["out"] for c in range(N_CORES)],
                          axis=0)


# revision 100
# speedup vs baseline: 1.3649x; 1.1171x over previous
"""Trainium2 Bass kernel for BlockDiagonalACDC — depth-2 DCT factorization.

out = riffle(idct2(gconv(dct2(gconv(x, A)), D))) + bias.

The 4096-point DCT-II is factored two levels deep:
  L1 fold:  u = f[:2048]+rev(f[2048:]), v = f[:2048]-rev(f[2048:])
  L2 fold:  ulo/uhi from u;  Givens rotations (p, qt) from v
  leaves:   four dense 1024x1024 matmuls (DCT2/DCT4/DCT2/DCT2)
  lift:     odd-k outputs are +-1 pairs of leaf2/leaf3 outputs, folded with
            gconv(D) and the idct input diag into one middle matrix Meff
            (applied as ~140 sparse 128x128 blocks).
  inverse:  transposed leaves (data-stationary), then transposed rotations /
            folds on [batch, j] tiles with broadcast coeff rows; riffle and
            the final reversal land as 8 contiguous 512-wide output runs.

z1 orientation trick: gconvA lhsT column-reversed for tiles 8-15/24-31 so all
fold/rotation pairings are partition-aligned elementwise DVE ops.

Sharding: pure data parallel, 2048 batch rows per core on 8 cores.
"""

import numpy as np
import ml_dtypes

import concourse.bacc as bacc
import concourse.mybir as mybir
from concourse.tile import TileContext
from concourse.bass_utils import run_bass_kernel_spmd
from concourse.masks import make_identity

N_BATCH, D_FEAT, GROUPS = 16384, 4096, 32
N_CORES = 8
N_SHARD = N_BATCH // N_CORES      # 2048 rows per core
CHUNK = 512
N_CHUNKS = N_SHARD // CHUNK       # 4
FTILES = D_FEAT // 128            # 32
N, H, Q = 4096, 2048, 1024

_BF16 = mybir.dt.bfloat16
_F32 = mybir.dt.float32
_MUL = mybir.AluOpType.mult
_ADD = mybir.AluOpType.add

# inverse output runs (512 wide): [strip][slot fa..fd] -> (start, reversed)
# strip 0 = even j' (asc), strip 1 = odd j' (colperm order)
RUN = [[(0, False), (3584, True), (2560, True), (1024, False)],
       [(2048, False), (1536, True), (512, True), (3072, False)]]
QW = 512
NSTRIP = 2


def _dct2m(n):
    k = np.arange(n)[:, None]; j = np.arange(n)[None, :]
    return np.cos(np.pi * k * (2 * j + 1) / (2 * n))


def _dct4m(n):
    m = np.arange(n)[:, None]; j = np.arange(n)[None, :]
    return np.cos(np.pi * (2 * m + 1) * (2 * j + 1) / (4 * n))


def _dst_sign():
    h = 8
    r = np.arange(h)[:, None]; j = np.arange(h)[None, :]
    d2 = np.cos(np.pi * r * (2 * j + 1) / (2 * h))
    s2 = np.sin(np.pi * (r + 1) * (2 * j + 1) / (2 * h))
    q = np.random.RandomState(0).randn(h)
    qt = q * np.where(np.arange(h) % 2 == 0, 1.0, -1.0)
    b = s2 @ q
    return 1.0 if np.allclose(b, (d2 @ qt)[::-1]) else -1.0


def _host_static():
    """A/D/bias-independent constants."""
    dst_sign = _dst_sign()
    # leaf3 col r holds B[r-1] = dst_sign*2*dct2(qt)[1024-r] (col 0: B[1023])
    # so lift pairs (leaf2 col r, leaf3 col r) are tile-aligned
    l3rows = np.r_[0, np.arange(1023, 0, -1)]
    leaf = [2.0 * _dct2m(Q), 2.0 * _dct4m(Q), 2.0 * _dct2m(Q),
            2.0 * _dct2m(Q)[l3rows]]

    alpha = (2 * np.arange(Q) + 1) * np.pi / (4 * H)
    rc, rs = np.cos(alpha), np.sin(alpha)
    sgn = np.where(np.arange(Q) % 2 == 0, 1.0, -1.0)
    qa, qb = -sgn * rs, sgn * rc

    # lift table: k -> up to 2 (leaf-flat index a, sign)
    A1 = np.zeros(N, np.int64); S1 = np.zeros(N)
    A2 = np.zeros(N, np.int64); S2 = np.zeros(N)
    for r in range(Q):
        A1[4 * r], S1[4 * r] = 0 * Q + r, 1.0
        A1[4 * r + 2], S1[4 * r + 2] = 1 * Q + r, 1.0
    A1[1], S1[1] = 2 * Q + 0, 1.0
    for r in range(1, Q):
        # leaf3 col r = B[r-1] (dst_sign folded into lift signs below)
        A1[4 * r - 1], S1[4 * r - 1] = 2 * Q + r, 1.0
        A2[4 * r - 1], S2[4 * r - 1] = 3 * Q + r, -dst_sign
        A1[4 * r + 1], S1[4 * r + 1] = 2 * Q + r, 1.0
        A2[4 * r + 1], S2[4 * r + 1] = 3 * Q + r, dst_sign
    A1[4 * Q - 1], S1[4 * Q - 1] = 3 * Q + 0, -dst_sign

    # fwd leaf weights: fwdw[L, m, i, tau, c] = leaf[L][128m+c, 128tau+i]
    fwdw = np.stack([
        M.T.reshape(8, 128, 8, 128).transpose(2, 1, 0, 3) for M in leaf
    ]).astype(ml_dtypes.bfloat16)

    # inverse: stage-col parity permutation, per strip
    colperm = np.concatenate([np.arange(0, Q, 2), np.arange(1, Q, 2)])
    # invw[strip, wave, i, b, kt, c] =
    #     (leaf[2*wave+b]/2)[128kt+i, colperm[512*strip+c]]
    pieces = [(M / 2.0)[:, colperm].reshape(8, 128, 2, 512) for M in leaf]
    invw = np.zeros((2, 2, 128, 2, 8, 512), np.float32)
    for L in range(4):
        # pieces[L][kt, i, s, c] -> invw[s, L//2, i, L%2, kt, c]
        invw[:, L // 2, :, L % 2, :, :] = pieces[L].transpose(2, 1, 0, 3)
    invw = invw.astype(ml_dtypes.bfloat16)

    # fwd rotation per-partition coeffs cfw[p, cf, t'] (cf: c,s,qa,qb)
    cfw = np.stack([rc, rs, qa, qb]).reshape(4, 8, 128).transpose(2, 0, 1)
    cfw = np.ascontiguousarray(cfw).astype(np.float32)

    # inverse rotation free-dim coeffs ciw[p, cf, strip, c] (broadcast rows)
    ci = np.stack([rc, rs, qa, qb])[:, colperm].reshape(4, 2, 512)
    ciw = np.broadcast_to(ci, (128, 4, 2, 512)).astype(ml_dtypes.bfloat16)
    ciw = np.ascontiguousarray(ciw)

    # Meff block sparsity pattern (structure only): big blocks are K=128
    # matmuls; sparse spill blocks decompose into <=32-row pieces (K=32).
    rng = np.random.RandomState(0)
    Mp = _build_meff(rng.randn(GROUPS, 128, 128), A1, S1, A2, S2)
    big = []     # (kt, mt)
    small = []   # (kt, mt, w): 32-row window w (w=3 runs as K=64 from base 64)
    for kt in range(32):
        for mt in range(32):
            blk = Mp[kt * 128:(kt + 1) * 128, mt * 128:(mt + 1) * 128]
            rows = np.where(np.abs(blk).max(axis=1) > 1e-12)[0]
            if len(rows) == 0:
                continue
            if len(rows) > 48:
                big.append((kt, mt))
                continue
            for w in sorted(set(int(r) // 32 for r in rows)):
                small.append((kt, mt, w))
    # pack small pieces into tile columns by window class; window 3 runs as
    # K=64 from base 64 so it must not share a column with a window-2 piece.
    cls = {w: [p for p in small if p[2] == w] for w in range(4)}
    ncol = max(len(cls[0]), len(cls[1]), len(cls[2]) + len(cls[3]), 1)
    packed = []  # (kt, mt, w, col)
    for w in (0, 1):
        for j, (kt, mt, _) in enumerate(cls[w]):
            packed.append((kt, mt, w, j))
    for j, (kt, mt, _) in enumerate(cls[2]):
        packed.append((kt, mt, 2, j))
    for j, (kt, mt, _) in enumerate(cls[3]):
        packed.append((kt, mt, 3, len(cls[2]) + j))

    return dict(leaf=leaf, fwdw=fwdw, invw=invw, cfw=cfw, ciw=ciw,
                colperm=colperm, lift=(A1, S1, A2, S2), big=big,
                packed=packed, ncol=ncol, rot=(rc, rs, qa, qb))


def _build_meff(D, A1, S1, A2, S2):
    w = np.ones(N); w[0] = 0.5
    wN = w / N
    M = np.zeros((N, N))
    for g in range(GROUPS):
        kk = np.arange(128) + 128 * g
        blk = D[g].T * wN[kk][None, :]        # [i, o]
        for (ai, si) in ((A1[kk], S1[kk]), (A2[kk], S2[kk])):
            for (ao, so) in ((A1[kk], S1[kk]), (A2[kk], S2[kk])):
                contrib = (si[:, None] * so[None, :]) * blk
                np.add.at(M, (ai[:, None], ao[None, :]), contrib)
    return M


def _build_program(reps=1):
    st = _prep()
    nbig = len(st["big"])
    nsmt = st["ncol"]
    nc = bacc.Bacc()
    # host-pre-transposed bf16 input: xs[ftile, part, batchrow]
    xs = nc.dram_tensor("xs", (FTILES, 128, N_SHARD), _BF16,
                        kind="ExternalInput")
    # 32 base lhsT tiles + negated copies of tiles 16..31 (slots 32..47)
    atv = nc.dram_tensor("atv", (128, FTILES + 16, 128), _BF16,
                         kind="ExternalInput")
    fwdw = nc.dram_tensor("fwdw", (4, 8, 128, 8, 128), _BF16,
                          kind="ExternalInput")
    meffw = nc.dram_tensor("meffw", (128, nbig, 128), _BF16,
                           kind="ExternalInput")
    smallw = nc.dram_tensor("smallw", (128, max(nsmt, 1), 128), _BF16,
                            kind="ExternalInput")
    invw = nc.dram_tensor("invw", (2, 2, 128, 2, 8, 512), _BF16,
                          kind="ExternalInput")
    cfw = nc.dram_tensor("cfw", (128, 4, 8), _F32, kind="ExternalInput")
    ciw = nc.dram_tensor("ciw", (128, 4, 2, 512), _BF16, kind="ExternalInput")
    bstw = nc.dram_tensor("bstw", (1, 4, 2, 512), _BF16,
                          kind="ExternalInput")
    out = nc.dram_tensor("out", (N_SHARD, D_FEAT), _F32, kind="ExternalOutput")

    with TileContext(nc) as tc:
        with (
            tc.tile_pool(name="const", bufs=1) as constp,
            tc.tile_pool(name="stage", bufs=3) as stagep,
            tc.tile_pool(name="xtp", bufs=13) as xtp,
            tc.tile_pool(name="fwp", bufs=4) as fwp,
            tc.tile_pool(name="mfp", bufs=2) as mfp,
            tc.tile_pool(name="ivp", bufs=2) as ivp,
            tc.tile_pool(name="tmp", bufs=7) as tmpp,
            tc.tile_pool(name="ost", bufs=6) as ostp,
            tc.tile_pool(name="mm_ps", bufs=4, space="PSUM") as mmp,
            tc.tile_pool(name="st_ps", bufs=4, space="PSUM") as stp,
        ):
            ident = constp.tile([128, 128], _BF16, tag="ident")
            make_identity(nc, ident[:])
            atv_t = constp.tile([128, FTILES + 16, 128], _BF16, tag="atv")
            nc.scalar.dma_start(atv_t[:], atv[:])
            small_t = constp.tile([128, max(nsmt, 1), 128], _BF16, tag="smeff")
            nc.scalar.dma_start(small_t[:], smallw[:])
            cf_t = constp.tile([128, 4, 8], _F32, tag="cf")
            nc.scalar.dma_start(cf_t[:], cfw[:])
            ci_t = constp.tile([128, 4, 2, 512], _BF16, tag="ci")
            nc.scalar.dma_start(ci_t[:], ciw[:])
            bst_t = constp.tile([1, 4, 2, 512], _BF16, tag="bst")
            nc.scalar.dma_start(bst_t[:], bstw[:])
            ones1 = constp.tile([1, 128], _BF16, tag="ones1")
            nc.gpsimd.memset(ones1[:], 1.0)

            # middle-matmul emission lists per output tile; big blocks are
            # stored in meffw sorted by mt (kernel() packs them identically)
            by_mt = {mt: [] for mt in range(32)}
            off = 0
            mt_off = {}
            for mt in range(32):
                mt_off[mt] = off
                for (kt, m2) in st["big"]:
                    if m2 == mt:
                        by_mt[mt].append(("big", off, kt, 0))
                        off += 1
            for (kt, mt, w, col) in st["packed"]:
                by_mt[mt].append(("small", col, kt, w))

            def cp(i, dst, src):
                # PSUM->SBUF copies alternate Act/DVE (Pool is slow)
                if i % 2 == 0:
                    nc.scalar.copy(dst, src)
                else:
                    nc.vector.tensor_copy(dst, src)

            def veng(i):
                return nc.gpsimd if i % 4 == 3 else nc.vector

            def sl(arr, t):
                return arr[:, t * CHUNK:(t + 1) * CHUNK]

            def emit_front(ci_):
                """xT tile loads + PE L1 partials + DVE L2 stages -> s2."""
                r0 = ci_ * CHUNK
                s2 = stagep.tile([128, FTILES * CHUNK], _BF16, tag="stage")
                for t in range(8):
                    # (lhs slot, xT tile) pairs; slots 32..47 hold -atv[16..31]
                    combos2 = (
                        ((t, t), (31 - t, 31 - t)),             # a1
                        ((15 - t, 15 - t), (16 + t, 16 + t)),   # a2
                        ((t, t), (47 - t, 31 - t)),             # b1
                        ((15 - t, 15 - t), (32 + t, 16 + t)),   # b2
                    )
                    xts = {}
                    for s in (t, 31 - t, 15 - t, 16 + t):
                        xt = xtp.tile([128, CHUNK], _BF16, tag="xt")
                        nc.sync.dma_start(xt[:], xs[s, :, r0:r0 + CHUNK])
                        xts[s] = xt
                    parts = []
                    for pi, pair in enumerate(combos2):
                        ps = mmp.tile([128, CHUNK], _F32, tag="mm")
                        for mi, (lslot, rtile) in enumerate(pair):
                            nc.tensor.matmul(
                                ps[:], atv_t[:, lslot, :], xts[rtile][:],
                                start=(mi == 0), stop=(mi == 1))
                        pt = tmpp.tile([128, CHUNK], _BF16, tag="tmp")
                        cp(pi, pt[:], ps[:])
                        parts.append(pt)
                    a1, a2, b1, b2 = parts
                    nc.vector.tensor_add(sl(s2, t), a1[:], a2[:])
                    nc.vector.tensor_sub(sl(s2, 8 + t), a1[:], a2[:])
                    # p = c*b1 + s*b2 ; qt = qa*b1 + qb*b2
                    tm = tmpp.tile([128, CHUNK], _BF16, tag="tmp")
                    nc.gpsimd.tensor_scalar(
                        tm[:], b1[:], cf_t[:, 0, t:t + 1], None, _MUL)
                    nc.vector.scalar_tensor_tensor(
                        sl(s2, 16 + t), b2[:], cf_t[:, 1, t:t + 1],
                        tm[:], _MUL, _ADD)
                    tm2 = tmpp.tile([128, CHUNK], _BF16, tag="tmp")
                    nc.gpsimd.tensor_scalar(
                        tm2[:], b1[:], cf_t[:, 2, t:t + 1], None, _MUL)
                    nc.vector.scalar_tensor_tensor(
                        sl(s2, 24 + t), b2[:], cf_t[:, 3, t:t + 1],
                        tm2[:], _MUL, _ADD)
                return s2

            def emit_back(ci_, s2):
                """fwd leaves then middle matmuls -> z3."""
                z2 = stagep.tile([128, FTILES * CHUNK], _BF16, tag="stage")
                for L in range(4):
                    for m in range(8):
                        fw = fwp.tile([128, 8, 128], _BF16, tag="fw")
                        nc.sync.dma_start(fw[:], fwdw[L, m])
                        ps = mmp.tile([128, CHUNK], _F32, tag="mm")
                        for tau in range(8):
                            nc.tensor.matmul(
                                ps[:], fw[:, tau, :], sl(s2, 8 * L + tau),
                                start=(tau == 0), stop=(tau == 7))
                        cp(m, sl(z2, 8 * L + m), ps[:])

                z3 = stagep.tile([128, FTILES * CHUNK], _BF16, tag="stage")
                for mt in range(32):
                    lst = by_mt[mt]
                    nb = sum(1 for e in lst if e[0] == "big")
                    mtile = mfp.tile([128, max(nb, 1), 128], _BF16, tag="mf")
                    if nb:
                        nc.sync.dma_start(
                            mtile[:, 0:nb, :],
                            meffw[:, mt_off[mt]:mt_off[mt] + nb, :])
                    ps = mmp.tile([128, CHUNK], _F32, tag="mm")
                    for li, (kind, bi, kt, w) in enumerate(lst):
                        if kind == "big":
                            lhsT = mtile[:, bi - mt_off[mt], :]
                            rhs = sl(z2, kt)
                        else:
                            b0 = 64 if w == 3 else 32 * w
                            kk = 64 if w == 3 else 32
                            lhsT = small_t[b0:b0 + kk, bi, :]
                            rhs = z2[b0:b0 + kk,
                                     kt * CHUNK:(kt + 1) * CHUNK]
                        nc.tensor.matmul(
                            ps[:], lhsT, rhs,
                            start=(li == 0), stop=(li == len(lst) - 1))
                    cp(mt, sl(z3, mt), ps[:])
                return z3

            def emit_inverse(ci_, z3):
                r0 = ci_ * CHUNK
                for strip in range(NSTRIP):
                    ivw = []
                    for wave in range(2):
                        iv = ivp.tile([128, 2, 8, QW], _BF16, tag="iv")
                        nc.sync.dma_start(iv[:], invw[strip, wave])
                        ivw.append(iv)
                    for nt in range(CHUNK // 128):
                        sb = []   # stage-space tiles ulo,uhi,p,qt bf16
                        for wave in range(2):
                            pss = []
                            for b in range(2):
                                L = 2 * wave + b
                                ps = stp.tile([128, QW], _F32, tag="st")
                                for kt in range(8):
                                    nc.tensor.matmul(
                                        ps[:],
                                        z3[:, (8 * L + kt) * CHUNK + nt * 128:
                                           (8 * L + kt) * CHUNK + (nt + 1) * 128],
                                        ivw[wave][:, b, kt, :],
                                        start=(kt == 0), stop=False)
                                nc.tensor.matmul(
                                    ps[:], ones1[:], bst_t[0:1, L, strip, :],
                                    start=False, stop=True)
                                pss.append(ps)
                            for b in range(2):
                                # Act queue only: must not sit behind DVE
                                # stage ops of the next chunk
                                tt = ostp.tile([128, QW], _BF16, tag="ost")
                                nc.scalar.copy(tt[:], pss[b][:])
                                sb.append(tt)
                        ulo, uhi, pp, qq = sb
                        u_lo = tmpp.tile([128, QW], _BF16, tag="tmp")
                        u_hi = tmpp.tile([128, QW], _BF16, tag="tmp")
                        nc.vector.tensor_add(u_lo[:], ulo[:], uhi[:])
                        nc.vector.tensor_sub(u_hi[:], ulo[:], uhi[:])
                        m1 = tmpp.tile([128, QW], _BF16, tag="tmp")
                        m2 = tmpp.tile([128, QW], _BF16, tag="tmp")
                        v_lo = tmpp.tile([128, QW], _BF16, tag="tmp")
                        v_hi = tmpp.tile([128, QW], _BF16, tag="tmp")
                        # v_lo = c*p + qa*qt ; v_hi = s*p + qb*qt
                        nc.vector.tensor_mul(m1[:], pp[:], ci_t[:, 0, strip, :])
                        nc.gpsimd.tensor_mul(m2[:], qq[:], ci_t[:, 2, strip, :])
                        nc.vector.tensor_add(v_lo[:], m1[:], m2[:])
                        m3 = tmpp.tile([128, QW], _BF16, tag="tmp")
                        m4 = tmpp.tile([128, QW], _BF16, tag="tmp")
                        nc.gpsimd.tensor_mul(m3[:], pp[:], ci_t[:, 1, strip, :])
                        nc.vector.tensor_mul(m4[:], qq[:], ci_t[:, 3, strip, :])
                        nc.vector.tensor_add(v_hi[:], m3[:], m4[:])
                        rows = slice(r0 + nt * 128, r0 + (nt + 1) * 128)
                        combos = [(u_lo, v_lo, 1), (u_lo, v_lo, -1),
                                  (u_hi, v_hi, 1), (u_hi, v_hi, -1)]
                        for si, (ua, va, sg) in enumerate(combos):
                            start, rev = RUN[strip][si]
                            ot = ostp.tile([128, QW], _BF16, tag="ost")
                            dst = ot[:, ::-1] if rev else ot[:]
                            if sg > 0:
                                veng(si).tensor_add(dst, ua[:], va[:])
                            else:
                                veng(si).tensor_sub(dst, ua[:], va[:])
                            nc.gpsimd.dma_start(
                                out[rows, start:start + QW], ot[:])

            rep_ctx = tc.For_i(0, reps, 1) if reps > 1 else None
            if rep_ctx is not None:
                rep_ctx.__enter__()
            # software pipeline: the inverse of chunk c-1 is emitted between
            # the front (partials+stages) and back (fwd+middle) of chunk c,
            # so its PE work covers the stage-DVE latency.
            z3_prev = None
            for ci_ in range(N_CHUNKS):
                s2 = emit_front(ci_)
                if z3_prev is not None:
                    emit_inverse(ci_ - 1, z3_prev)
                z3_prev = emit_back(ci_, s2)
            emit_inverse(N_CHUNKS - 1, z3_prev)
            if rep_ctx is not None:
                rep_ctx.__exit__(None, None, None)
    nc.finalize()
    return nc


_CACHE = {}


def _prep():
    if "static" not in _CACHE:
        _CACHE["static"] = _host_static()
    return _CACHE["static"]


def kernel(x, A, D, bias):
    st = _prep()
    if "nc" not in _CACHE:
        _CACHE["nc"] = _build_program()
    nc = _CACHE["nc"]

    # gconvA lhsT with orientation folding: slot t rev for 8-15, 24-31;
    # slots 32..47 = negated copies of slots 16..31 (for b1/b2 partials)
    At = np.asarray(A, dtype=np.float32)
    atv = np.zeros((128, FTILES + 16, 128), np.float32)
    for t in range(FTILES):
        W = At[t].T                       # lhsT [i, o]
        if (8 <= t < 16) or (24 <= t < 32):
            W = W[:, ::-1]
        atv[:, t, :] = W
    atv[:, 32:48, :] = -atv[:, 16:32, :]
    atv = atv.astype(ml_dtypes.bfloat16)

    Meff = _build_meff(np.asarray(D, dtype=np.float64), *st["lift"])
    meffw = np.zeros((128, len(st["big"]), 128), np.float32)
    off = 0
    for mt in range(32):     # sorted by mt to match streaming order
        for (kt, m2) in st["big"]:
            if m2 == mt:
                meffw[:, off, :] = Meff[kt * 128:(kt + 1) * 128,
                                        mt * 128:(mt + 1) * 128]
                off += 1
    meffw = meffw.astype(ml_dtypes.bfloat16)
    smallw = np.zeros((128, st["ncol"], 128), np.float32)
    for (kt, mt, w, col) in st["packed"]:
        smallw[32 * w:32 * w + 32, col, :] = \
            Meff[kt * 128 + 32 * w:kt * 128 + 32 * w + 32,
                 mt * 128:(mt + 1) * 128]
    smallw = smallw.astype(ml_dtypes.bfloat16)

    # bias pre-image in stage space (device injects at leaf psums via K=1)
    rc, rs, qa, qb = st["rot"]
    bias_v = np.asarray(bias, dtype=np.float64).reshape(-1)
    j = np.arange(Q)
    def rif(f):
        return (f % 2) * 2048 + f // 2
    t1 = bias_v[rif(j)]
    t2 = bias_v[rif(4095 - j)]
    t3 = bias_v[rif(2047 - j)]
    t4 = bias_v[rif(2048 + j)]
    U1, V1 = (t1 + t2) / 2, (t1 - t2) / 2
    U2, V2 = (t3 + t4) / 2, (t3 - t4) / 2
    b_ulo, b_uhi = (U1 + U2) / 2, (U1 - U2) / 2
    b_p = rc * V1 + rs * V2
    b_qt = qa * V1 + qb * V2
    bst = np.stack([b_ulo, b_uhi, b_p, b_qt])          # [4, 1024]
    bst = bst[:, st["colperm"]].reshape(4, 2, 512)[None]
    bst = np.ascontiguousarray(bst.astype(ml_dtypes.bfloat16))

    x = np.ascontiguousarray(x, dtype=np.float32)
    in_maps = []
    for c in range(N_CORES):
        shard = x[c * N_SHARD:(c + 1) * N_SHARD]
        xs_t = np.ascontiguousarray(shard.T).reshape(
            FTILES, 128, N_SHARD).astype(ml_dtypes.bfloat16)
        in_maps.append({
            "xs": xs_t, "atv": atv, "fwdw": st["fwdw"], "meffw": meffw,
            "smallw": smallw, "invw": st["invw"], "cfw": st["cfw"],
            "ciw": st["ciw"], "bstw": bst,
        })
    res = run_bass_kernel_spmd(nc, in_maps, core_ids=list(range(N_CORES)))
    return np.concatenate([res.results[c]["out"] for c in range(N_CORES)],
                          axis=0)
